# revision 5
# baseline (speedup 1.0000x reference)
"""Two-layer GraphSAGE (mean aggregation) on 8 Trainium2 NeuronCores.

Strategy (sharding_hint: shard nodes + edges by destination, replicate
weights, exchange source features for cross-partition edges):

  * Nodes are sharded contiguously across 8 cores (12500 each); edges are
    partitioned by destination shard and sorted into 128-node destination
    windows on the host.
  * Layer-1 aggregation uses linearity: segsum(x[src]) @ W1_l, so raw x rows
    are gathered (x is replicated to every core at upload time - no exchange
    needed for layer 1).
  * Per 128-edge slice, a one-hot matrix M[e,n] = (dst_local[e]==n) is built
    on-device (iota + is_equal) and the segment sum becomes a PE matmul
    accumulated in PSUM: S1[n,f] += M^T @ G.
  * Layer 2 transforms first (P2 = h @ W2_l, 64 cols instead of 128), then
    the P2 shards are exchanged with 4 chunked AllGathers overlapped with
    phase 1, and the second aggregation gathers P2 rows.
  * Rows are fetched with gpsimd dma_gather (int16 indices => the gather
    tables are split into <=32767-row blocks; each window's edges are
    grouped by source block on the host).

Self-contained: hardcodes the problem shapes from the task spec.
"""

import numpy as np

# ---------------------------------------------------------------- config

IN_CH, HIDDEN, OUT_CH = 128, 128, 64
N_NODES, N_EDGES = 100000, 1600000
NCORES = 8
P = 128                      # partitions / window size
L1_RANGE = 25000             # L1 gather block size (int16 limit)


def _derive_cfg(n_nodes):
    shard = n_nodes // NCORES
    nwin = (shard + P - 1) // P
    nchunk = 4 if nwin >= 4 else 1
    chunk_wins = (nwin + nchunk - 1) // nchunk
    # per-core rows per chunk
    chunk_rows = []
    for c in range(nchunk):
        lo = c * chunk_wins * P
        hi = min((c + 1) * chunk_wins * P, shard)
        chunk_rows.append(max(hi - lo, 0))
    ngrp1 = (n_nodes + L1_RANGE - 1) // L1_RANGE
    return dict(shard=shard, nwin=nwin, nchunk=nchunk, chunk_wins=chunk_wins,
                chunk_rows=chunk_rows, ngrp1=ngrp1)


def _round16(x):
    return (x + 15) // 16 * 16


# ---------------------------------------------------------------- host prep

def _preprocess(x, edge_index, cfg):
    n = x.shape[0]
    shard, nwin, nchunk = cfg["shard"], cfg["nwin"], cfg["nchunk"]
    chunk_wins, chunk_rows, ngrp1 = cfg["chunk_wins"], cfg["chunk_rows"], cfg["ngrp1"]

    src = np.asarray(edge_index[0], dtype=np.int64)
    dst = np.asarray(edge_index[1], dtype=np.int64)
    deg = np.bincount(dst, minlength=n).astype(np.float32)
    deg_inv = np.where(deg > 0, np.float32(1.0) / np.maximum(deg, 1.0), 0.0
                       ).astype(np.float32)

    core = dst // shard
    local = dst % shard
    win = local // P
    dstl = (local % P).astype(np.int32)

    # L1 grouping by source range block (block stride L1_RANGE, +1 zero row)
    g1 = np.minimum(src // L1_RANGE, ngrp1 - 1)
    l1loc = (src - g1 * L1_RANGE).astype(np.int32)      # < L1_RANGE+? (last blk)
    l1_blk_rows = [min(L1_RANGE, n - q * L1_RANGE) for q in range(ngrp1)]

    # L2 grouping by source chunk block in the AllGather layout
    csz = chunk_wins * P
    c2 = np.minimum((src % shard) // csz, nchunk - 1)
    # position within block c: (core of src)*chunk_rows[c] + offset in chunk
    l2loc = ((src // shard) * np.array(chunk_rows)[c2]
             + (src % shard) - c2 * csz).astype(np.int32)
    l2_blk_rows = [NCORES * r for r in chunk_rows]      # rows per block (excl zero)

    # static padded counts: max over cores per (win, grp), rounded to 16
    def counts(grp, ngrp):
        key = (core * nwin + win) * ngrp + grp
        cnt = np.bincount(key, minlength=NCORES * nwin * ngrp)
        cnt = cnt.reshape(NCORES, nwin, ngrp)
        return _round16(cnt.max(axis=0))                # [nwin, ngrp]

    T1 = counts(g1, ngrp1)
    T2 = counts(c2, nchunk)
    S1 = np.maximum((T1 + P - 1) // P, 0)               # slices per (win,grp)
    S2 = np.maximum((T2 + P - 1) // P, 0)

    # per-core per-layer packed arrays
    def pack(grp, ngrp, T, S, loc, zero_rows):
        """Build idx16 [128, sumT/16] and dstl_f32 [128, sumS] per core."""
        sumT = int(T.sum())
        sumS = int(S.sum())
        idx_all = np.zeros((NCORES, 16, sumT // 16), np.int16)
        dst_all = np.full((NCORES, P, sumS), 999.0, np.float32)
        order = np.lexsort((grp, win, core))
        so, go, wo, co = (a[order] for a in (src, grp, win, core))
        lo_o, dl_o = loc[order], dstl[order]
        # boundaries per (core, win, grp)
        key = (co * nwin + wo) * ngrp + go
        cnt = np.bincount(key, minlength=NCORES * nwin * ngrp
                          ).reshape(NCORES, nwin, ngrp)
        starts = np.zeros_like(cnt)
        pos = 0
        # column offsets of each (win, grp) in the packed arrays (shared)
        colT = np.concatenate([[0], np.cumsum(T.ravel())])[:-1].reshape(T.shape)
        colS = np.concatenate([[0], np.cumsum(S.ravel())])[:-1].reshape(S.shape)
        e0 = 0
        for ci in range(NCORES):
            for w in range(nwin):
                for q in range(ngrp):
                    k = cnt[ci, w, q]
                    ids = lo_o[e0:e0 + k]
                    dls = dl_o[e0:e0 + k]
                    e0 += k
                    t = int(T[w, q])
                    if t == 0:
                        continue
                    buf = np.full(t, zero_rows[q], np.int32)
                    buf[:k] = ids
                    base = int(colT[w, q]) // 16
                    idx_all[ci, :, base:base + t // 16] = (
                        buf.reshape(t // 16, 16).T)
                    db = np.full(((t + P - 1) // P) * P, 999.0, np.float32)
                    db[:k] = dls
                    sbase = int(colS[w, q])
                    ns = (t + P - 1) // P
                    dst_all[ci, :, sbase:sbase + ns] = (
                        db.reshape(ns, P).T)
        assert e0 == len(order)
        # replicate idx 16-partition pattern to 128 partitions
        idx_rep = np.tile(idx_all, (1, 8, 1))
        return idx_rep, dst_all, colT, colS, sumT, sumS

    zr1 = l1_blk_rows                          # zero row index per L1 block
    zr2 = l2_blk_rows                          # zero row index per L2 block
    idx1, dst1, colT1, colS1, sumT1, sumS1 = pack(g1, ngrp1, T1, S1, l1loc, zr1)
    idx2, dst2, colT2, colS2, sumT2, sumS2 = pack(c2, nchunk, T2, S2, l2loc, zr2)

    # x table with per-block zero row: block q rows [q*(rows+1) ... ]
    xblocks = []
    for q in range(ngrp1):
        xb = x[q * L1_RANGE: q * L1_RANGE + l1_blk_rows[q]]
        xblocks.append(np.concatenate([xb, np.zeros((1, x.shape[1]), np.float32)]))
    xdev = np.concatenate(xblocks, axis=0)
    l1_base = np.concatenate([[0], np.cumsum([b.shape[0] for b in xblocks])])[:-1]

    # per-core transposed shard + deg_inv layout
    xts, dinvs = [], []
    for ci in range(NCORES):
        xs = x[ci * shard:(ci + 1) * shard]
        pad = nwin * P - shard
        xts.append(np.concatenate(
            [xs, np.zeros((pad, x.shape[1]), np.float32)]).T.copy())
        dv = np.concatenate([deg_inv[ci * shard:(ci + 1) * shard],
                             np.zeros(pad, np.float32)])
        dinvs.append(dv.reshape(nwin, P).T.copy())

    meta = dict(T1=T1, T2=T2, S1=S1, S2=S2, colT1=colT1, colS1=colS1,
                colT2=colT2, colS2=colS2, sumT1=sumT1, sumS1=sumS1,
                sumT2=sumT2, sumS2=sumS2, l1_base=l1_base,
                l1_blk_rows=l1_blk_rows, l2_blk_rows=l2_blk_rows)
    data = dict(xdev=xdev, idx1=idx1, dst1=dst1, idx2=idx2, dst2=dst2,
                xts=xts, dinvs=dinvs)
    return meta, data


# ---------------------------------------------------------------- builder

def _build(cfg, meta):
    import concourse.bacc as bacc
    import concourse.mybir as mybir
    import concourse.tile as tile

    f32 = mybir.dt.float32
    shard, nwin, nchunk = cfg["shard"], cfg["nwin"], cfg["nchunk"]
    chunk_wins, chunk_rows, ngrp1 = cfg["chunk_wins"], cfg["chunk_rows"], cfg["ngrp1"]
    T1, T2, S1, S2 = meta["T1"], meta["T2"], meta["S1"], meta["S2"]
    colT1, colS1 = meta["colT1"], meta["colS1"]
    colT2, colS2 = meta["colT2"], meta["colS2"]
    l1_base = meta["l1_base"]
    l1_blk_rows, l2_blk_rows = meta["l1_blk_rows"], meta["l2_blk_rows"]
    S1w = S1.sum(axis=1)          # slices per window, layer 1
    S2w = S2.sum(axis=1)
    S1max, S2max = int(S1w.max()), int(S2w.max())
    xdev_rows = int(l1_base[-1] + l1_blk_rows[-1] + 1)

    # P2_full block offsets (each block followed by one zero row)
    p2_off = np.concatenate([[0], np.cumsum([r + 1 for r in l2_blk_rows])])
    p2_rows = int(p2_off[-1])

    nc = bacc.Bacc()
    dp = nc.declare_dram_parameter
    xdev = dp("xdev", [xdev_rows, IN_CH], f32, isOutput=False)
    xt = dp("xt", [P, nwin * P], f32, isOutput=False)
    idx1 = dp("idx1", [P, meta["sumT1"] // 16], mybir.dt.int16, isOutput=False)
    dst1 = dp("dst1", [P, meta["sumS1"]], f32, isOutput=False)
    idx2 = dp("idx2", [P, meta["sumT2"] // 16], mybir.dt.int16, isOutput=False)
    dst2 = dp("dst2", [P, meta["sumS2"]], f32, isOutput=False)
    dinv = dp("dinv", [P, nwin], f32, isOutput=False)
    w1l = dp("w1l", [IN_CH, HIDDEN], f32, isOutput=False)
    w1r = dp("w1r", [IN_CH, HIDDEN], f32, isOutput=False)
    w2l = dp("w2l", [HIDDEN, OUT_CH], f32, isOutput=False)
    w2r = dp("w2r", [HIDDEN, OUT_CH], f32, isOutput=False)
    b1c = dp("b1c", [P, 1], f32, isOutput=False)
    b2b = dp("b2b", [P, OUT_CH], f32, isOutput=False)
    iota = dp("iota", [P, P], f32, isOutput=False)
    ident = dp("ident", [P, P], f32, isOutput=False)
    y = dp("y", [shard, OUT_CH], f32, isOutput=True)

    p2_full = nc.dram_tensor("p2_full", [p2_rows, OUT_CH], f32,
                             addr_space="Shared")

    with tile.TileContext(nc) as tc:
        with (
            tc.tile_pool(name="const", bufs=1) as cb,
            tc.tile_pool(name="sb", bufs=3) as sb,
            tc.tile_pool(name="ps", bufs=2, space="PSUM") as ps,
            tc.tile_pool(name="psb", bufs=1, space="PSUM") as psb,
            tc.tile_pool(name="dram", bufs=1, space="DRAM") as dr,
        ):
            # ---- constants
            def cload(param, shape, tag):
                t = cb.tile(shape, f32, tag=tag)
                nc.sync.dma_start(out=t[:], in_=param[:])
                return t
            iota_t = cload(iota, [P, P], "c_iota")
            ident_t = cload(ident, [P, P], "c_ident")
            w1l_t = cload(w1l, [IN_CH, HIDDEN], "c_w1l")
            w1r_t = cload(w1r, [IN_CH, HIDDEN], "c_w1r")
            w2l_t = cload(w2l, [HIDDEN, OUT_CH], "c_w2l")
            w2r_t = cload(w2r, [HIDDEN, OUT_CH], "c_w2r")
            b1_t = cload(b1c, [P, 1], "c_b1")
            b2_t = cload(b2b, [P, OUT_CH], "c_b2")
            dinv_t = cload(dinv, [P, nwin], "c_dinv")
            r2_t = cb.tile([P, nwin * OUT_CH], f32)     # persistent R2
            zrow_t = cb.tile([P, OUT_CH], f32)
            nc.vector.memset(zrow_t[:], 0.0)

            # P2 chunk DRAM tiles (collective inputs)
            p2c = []
            for c in range(nchunk):
                p2c_tile = dr.tile([max(chunk_rows[c], 1), OUT_CH], f32,
                                   tag=f"p2c{c}")
                p2c.append(p2c_tile)

            # zero rows of p2_full (written once, before collectives run)
            for c in range(nchunk):
                zr = int(p2_off[c] + l2_blk_rows[c])
                nc.sync.dma_start(out=p2_full[zr:zr + 1, :], in_=zrow_t[:1, :])

            relu = mybir.ActivationFunctionType.Relu
            copyf = mybir.ActivationFunctionType.Copy

            # ---------------- phase 1 ----------------
            for w in range(nwin):
                n_w = min(shard - w * P, P)
                s1w = int(S1w[w])
                if s1w == 0:
                    continue
                # load idx/dstl/xt slices for this window
                it = sb.tile([P, int(T1[w].sum()) // 16], mybir.dt.int16, tag="it1")
                nc.sync.dma_start(
                    out=it[:], in_=idx1[:, int(colT1[w, 0]) // 16:
                                        (int(colT1[w, 0]) + int(T1[w].sum())) // 16])
                dt_ = sb.tile([P, s1w], f32, tag="dt1")
                nc.sync.dma_start(
                    out=dt_[:], in_=dst1[:, int(colS1[w, 0]):int(colS1[w, 0]) + s1w])
                xtw = sb.tile([P, P], f32, tag="xtw")
                nc.sync.dma_start(out=xtw[:], in_=xt[:, w * P:(w + 1) * P])

                # gather slab
                gat = sb.tile([P, S1max * IN_CH], f32, tag="g1")
                nc.vector.memset(gat[:, :s1w * IN_CH], 0.0)
                for q in range(ngrp1):
                    t_q = int(T1[w, q])
                    if t_q == 0:
                        continue
                    cq = (t_q + P - 1) // P
                    sbase = int((S1[w, :q]).sum())
                    ibase = int(colT1[w, q] - colT1[w, 0]) // 16
                    blo = int(l1_base[q])
                    nrows = l1_blk_rows[q] + 1
                    nc.gpsimd.dma_gather(
                        out_ap=gat[:, sbase * IN_CH:(sbase + cq) * IN_CH]
                        .rearrange("p (c e) -> p c e", e=IN_CH),
                        in_ap=xdev[blo:blo + nrows, :],
                        idxs_ap=it[:, ibase:ibase + t_q // 16],
                        num_idxs=t_q,
                        num_idxs_reg=t_q,
                        elem_size=IN_CH,
                        single_packet=False,
                    )

                # aggregation matmuls
                psum1 = ps.tile([P, IN_CH], f32, tag="ps1", space="PSUM")
                for g in range(s1w):
                    m = sb.tile([P, P], f32, tag="m1")
                    nc.vector.tensor_scalar(
                        out=m[:], in0=iota_t[:], scalar1=dt_[:, g:g + 1],
                        scalar2=None, op0=mybir.AluOpType.is_equal)
                    nc.tensor.matmul(
                        out=psum1[:], lhsT=m[:],
                        rhs=gat[:, g * IN_CH:(g + 1) * IN_CH],
                        start=(g == 0), stop=(g == s1w - 1))

                # T1 = deginv * S1   [n,f]
                t1sb = sb.tile([P, IN_CH], f32, tag="t1sb")
                nc.scalar.activation(out=t1sb[:], in_=psum1[:], func=copyf,
                                     scale=dinv_t[:, w:w + 1])
                # transpose -> [f,n]
                pst = psb.tile([P, P], f32, tag="pst", space="PSUM")
                nc.tensor.transpose(out=pst[:], in_=t1sb[:], identity=ident_t[:])
                t1t = sb.tile([P, P], f32, tag="t1t")
                nc.vector.tensor_copy(out=t1t[:], in_=pst[:])
                # hT = relu(W1l^T T1T + W1r^T XTw + b1)  [h,n]
                psum2 = psb.tile([P, P], f32, tag="ps2", space="PSUM")
                nc.tensor.matmul(out=psum2[:], lhsT=w1l_t[:], rhs=t1t[:],
                                 start=True, stop=False)
                nc.tensor.matmul(out=psum2[:], lhsT=w1r_t[:], rhs=xtw[:],
                                 start=False, stop=True)
                ht = sb.tile([P, P], f32, tag="ht")
                nc.scalar.activation(out=ht[:], in_=psum2[:], func=relu,
                                     bias=b1_t[:, :1])
                # P2 rows = h @ W2_l  [n,64]
                psum3 = psb.tile([P, OUT_CH], f32, tag="ps3", space="PSUM")
                nc.tensor.matmul(out=psum3[:], lhsT=ht[:], rhs=w2l_t[:],
                                 start=True, stop=True)
                p2sb = sb.tile([P, OUT_CH], f32, tag="p2sb")
                nc.scalar.activation(out=p2sb[:], in_=psum3[:], func=copyf)
                c = min(w // chunk_wins, nchunk - 1)
                r0 = w * P - c * chunk_wins * P
                nc.sync.dma_start(out=p2c[c][r0:r0 + n_w, :], in_=p2sb[:n_w, :])
                # R2 = h @ W2_r + b2  [n,64] persistent
                psum4 = psb.tile([P, OUT_CH], f32, tag="ps4", space="PSUM")
                nc.tensor.matmul(out=psum4[:], lhsT=ht[:], rhs=w2r_t[:],
                                 start=True, stop=True)
                nc.vector.tensor_add(out=r2_t[:, w * OUT_CH:(w + 1) * OUT_CH],
                                     in0=psum4[:], in1=b2_t[:])

                # chunk AllGather once its windows are done
                if (w + 1) % chunk_wins == 0 or w == nwin - 1:
                    if (w + 1) % chunk_wins == 0:
                        c_done = (w + 1) // chunk_wins - 1
                    else:
                        c_done = nchunk - 1
                    off = int(p2_off[c_done])
                    rows = l2_blk_rows[c_done]
                    nc.gpsimd.collective_compute(
                        "AllGather",
                        mybir.AluOpType.bypass,
                        replica_groups=[list(range(NCORES))],
                        ins=[p2c[c_done].opt()],
                        outs=[p2_full[off:off + rows, :]],
                    )

            # ---------------- phase 2 ----------------
            for w in range(nwin):
                n_w = min(shard - w * P, P)
                s2w = int(S2w[w])
                if s2w == 0:
                    # still must produce output rows (isolated nodes): y = R2
                    ysb = sb.tile([P, OUT_CH], f32, tag="ysb")
                    nc.vector.tensor_copy(
                        out=ysb[:], in_=r2_t[:, w * OUT_CH:(w + 1) * OUT_CH])
                    nc.sync.dma_start(out=y[w * P:w * P + n_w, :], in_=ysb[:n_w, :])
                    continue
                it = sb.tile([P, int(T2[w].sum()) // 16], mybir.dt.int16, tag="it2")
                nc.sync.dma_start(
                    out=it[:], in_=idx2[:, int(colT2[w, 0]) // 16:
                                        (int(colT2[w, 0]) + int(T2[w].sum())) // 16])
                dt_ = sb.tile([P, s2w], f32, tag="dt2")
                nc.sync.dma_start(
                    out=dt_[:], in_=dst2[:, int(colS2[w, 0]):int(colS2[w, 0]) + s2w])
                gat = sb.tile([P, S2max * OUT_CH], f32, tag="g2")
                nc.vector.memset(gat[:, :s2w * OUT_CH], 0.0)
                for q in range(nchunk):
                    t_q = int(T2[w, q])
                    if t_q == 0:
                        continue
                    cq = (t_q + P - 1) // P
                    sbase = int((S2[w, :q]).sum())
                    ibase = int(colT2[w, q] - colT2[w, 0]) // 16
                    off = int(p2_off[q])
                    nrows = l2_blk_rows[q] + 1
                    nc.gpsimd.dma_gather(
                        out_ap=gat[:, sbase * OUT_CH:(sbase + cq) * OUT_CH]
                        .rearrange("p (c e) -> p c e", e=OUT_CH),
                        in_ap=p2_full[off:off + nrows, :],
                        idxs_ap=it[:, ibase:ibase + t_q // 16],
                        num_idxs=t_q,
                        num_idxs_reg=t_q,
                        elem_size=OUT_CH,
                        single_packet=False,
                    )
                psum5 = ps.tile([P, OUT_CH], f32, tag="ps5", space="PSUM")
                for g in range(s2w):
                    m = sb.tile([P, P], f32, tag="m2")
                    nc.vector.tensor_scalar(
                        out=m[:], in0=iota_t[:], scalar1=dt_[:, g:g + 1],
                        scalar2=None, op0=mybir.AluOpType.is_equal)
                    nc.tensor.matmul(
                        out=psum5[:], lhsT=m[:],
                        rhs=gat[:, g * OUT_CH:(g + 1) * OUT_CH],
                        start=(g == 0), stop=(g == s2w - 1))
                ysb = sb.tile([P, OUT_CH], f32, tag="ysb")
                nc.scalar.activation(out=ysb[:], in_=psum5[:], func=copyf,
                                     scale=dinv_t[:, w:w + 1])
                nc.vector.tensor_add(out=ysb[:], in0=ysb[:],
                                     in1=r2_t[:, w * OUT_CH:(w + 1) * OUT_CH])
                nc.sync.dma_start(out=y[w * P:w * P + n_w, :], in_=ysb[:n_w, :])

    nc.compile()
    return nc


# ---------------------------------------------------------------- entry

_CACHE = {}


def kernel(x, edge_index, W1_l, W1_r, b1, W2_l, W2_r, b2):
    x = np.asarray(x, dtype=np.float32)
    edge_index = np.asarray(edge_index)
    cfg = _derive_cfg(x.shape[0])
    meta, data = _preprocess(x, edge_index, cfg)

    key = (x.shape, edge_index.shape)
    if key in _CACHE and _CACHE[key][1] == _meta_sig(meta):
        nc = _CACHE[key][0]
    else:
        nc = _build(cfg, meta)
        _CACHE[key] = (nc, _meta_sig(meta))

    in_maps = _make_inmaps(
        dict(W1_l=W1_l, W1_r=W1_r, b1=b1, W2_l=W2_l, W2_r=W2_r, b2=b2),
        meta, data)

    from concourse.bass_utils import run_bass_kernel_spmd
    r = run_bass_kernel_spmd(nc, in_maps, core_ids=list(range(NCORES)))
    out = np.concatenate([r.results[c]["y"] for c in range(NCORES)], axis=0)
    return out.astype(np.float32)


def _meta_sig(meta):
    return (int(meta["sumT1"]), int(meta["sumS1"]),
            int(meta["sumT2"]), int(meta["sumS2"]))


def _make_inmaps(inputs, meta, data):
    iota_v = np.tile(np.arange(P, dtype=np.float32), (P, 1))
    ident_v = np.eye(P, dtype=np.float32)
    common = dict(
        xdev=data["xdev"],
        w1l=np.asarray(inputs["W1_l"], np.float32),
        w1r=np.asarray(inputs["W1_r"], np.float32),
        w2l=np.asarray(inputs["W2_l"], np.float32),
        w2r=np.asarray(inputs["W2_r"], np.float32),
        b1c=np.asarray(inputs["b1"], np.float32).reshape(P, 1),
        b2b=np.tile(np.asarray(inputs["b2"], np.float32), (P, 1)),
        iota=iota_v, ident=ident_v,
    )
    in_maps = []
    for ci in range(NCORES):
        m = dict(common)
        m["xt"] = data["xts"][ci]
        m["dinv"] = data["dinvs"][ci]
        m["idx1"] = data["idx1"][ci]
        m["dst1"] = data["dst1"][ci]
        m["idx2"] = data["idx2"][ci]
        m["dst2"] = data["dst2"][ci]
        in_maps.append(m)
    return in_maps


# revision 15
# speedup vs baseline: 1.4681x; 1.4681x over previous
"""Two-layer GraphSAGE (mean aggregation) on 8 Trainium2 NeuronCores.

Strategy (sharding_hint: shard nodes + edges by destination, replicate
weights, exchange source features for cross-partition edges):

  * Nodes are sharded contiguously across 8 cores (12500 each); edges are
    partitioned by destination shard and sorted into 128-node destination
    windows on the host.
  * Layer-1 aggregation uses linearity: segsum(x[src]) @ W1_l, so raw x rows
    are gathered (x is replicated to every core at upload time - no exchange
    needed for layer 1).
  * Per 128-edge slice, a one-hot matrix M[e,n] = (dst_local[e]==n) is built
    on-device (iota + is_equal) and the segment sum becomes a PE matmul
    accumulated in PSUM: S1[n,f] += M^T @ G.
  * Layer 2 transforms first (P2 = h @ W2_l, 64 cols instead of 128), then
    the P2 shards are exchanged with 4 chunked AllGathers overlapped with
    phase 1, and the second aggregation gathers P2 rows.
  * Rows are fetched with gpsimd dma_gather (int16 indices => the gather
    tables are split into <=32767-row blocks; each window's edges are
    grouped by source block on the host).

Self-contained: hardcodes the problem shapes from the task spec.
"""

import numpy as np

# ---------------------------------------------------------------- config

IN_CH, HIDDEN, OUT_CH = 128, 128, 64
N_NODES, N_EDGES = 100000, 1600000
NCORES = 8
P = 128                      # partitions / window size
L1_RANGE = 25000             # L1 gather block size (int16 limit)


def _derive_cfg(n_nodes):
    shard = n_nodes // NCORES
    nwin = (shard + P - 1) // P
    nchunk = 4 if nwin >= 4 else 1
    chunk_wins = (nwin + nchunk - 1) // nchunk
    # per-core rows per chunk
    chunk_rows = []
    for c in range(nchunk):
        lo = c * chunk_wins * P
        hi = min((c + 1) * chunk_wins * P, shard)
        chunk_rows.append(max(hi - lo, 0))
    ngrp1 = (n_nodes + L1_RANGE - 1) // L1_RANGE
    return dict(shard=shard, nwin=nwin, nchunk=nchunk, chunk_wins=chunk_wins,
                chunk_rows=chunk_rows, ngrp1=ngrp1)


def _round16(x):
    return (x + 15) // 16 * 16


# ---------------------------------------------------------------- host prep

def _preprocess(x, edge_index, cfg):
    n = x.shape[0]
    shard, nwin, nchunk = cfg["shard"], cfg["nwin"], cfg["nchunk"]
    chunk_wins, chunk_rows, ngrp1 = cfg["chunk_wins"], cfg["chunk_rows"], cfg["ngrp1"]

    src = np.asarray(edge_index[0], dtype=np.int64)
    dst = np.asarray(edge_index[1], dtype=np.int64)
    deg = np.bincount(dst, minlength=n).astype(np.float32)
    deg_inv = np.where(deg > 0, np.float32(1.0) / np.maximum(deg, 1.0), 0.0
                       ).astype(np.float32)

    core = dst // shard
    local = dst % shard
    win = local // P
    dstl = (local % P).astype(np.int32)

    # L1 grouping by source range block (block stride L1_RANGE, +1 zero row)
    g1 = np.minimum(src // L1_RANGE, ngrp1 - 1)
    l1loc = (src - g1 * L1_RANGE).astype(np.int32)      # < L1_RANGE+? (last blk)
    l1_blk_rows = [min(L1_RANGE, n - q * L1_RANGE) for q in range(ngrp1)]

    # L2 grouping by source chunk block in the AllGather layout
    csz = chunk_wins * P
    c2 = np.minimum((src % shard) // csz, nchunk - 1)
    # position within block c: (core of src)*chunk_rows[c] + offset in chunk
    l2loc = ((src // shard) * np.array(chunk_rows)[c2]
             + (src % shard) - c2 * csz).astype(np.int32)
    l2_blk_rows = [NCORES * r for r in chunk_rows]      # rows per block (excl zero)

    # static padded counts: max over cores per (win, grp), rounded to 16
    def counts(grp, ngrp):
        key = (core * nwin + win) * ngrp + grp
        cnt = np.bincount(key, minlength=NCORES * nwin * ngrp)
        cnt = cnt.reshape(NCORES, nwin, ngrp)
        return _round16(cnt.max(axis=0))                # [nwin, ngrp]

    T1 = counts(g1, ngrp1)
    T2 = counts(c2, nchunk)
    S1 = np.maximum((T1 + P - 1) // P, 0)               # slices per (win,grp)
    S2 = np.maximum((T2 + P - 1) // P, 0)

    dg_edge = deg_inv[dst]                     # deginv of each edge's dst

    # per-core per-layer packed arrays
    def pack(grp, ngrp, T, S, loc, zero_rows):
        """Build idx16 [128, sumT/16], dstl_f32 and dg_f32 [128, sumS] per core."""
        sumT = int(T.sum())
        sumS = int(S.sum())
        idx_all = np.zeros((NCORES, 16, sumT // 16), np.int16)
        dst_all = np.full((NCORES, P, sumS), 999.0, np.float32)
        dg_all = np.zeros((NCORES, P, sumS), np.float32)
        order = np.lexsort((grp, win, core))
        so, go, wo, co = (a[order] for a in (src, grp, win, core))
        lo_o, dl_o = loc[order], dstl[order]
        dg_o = dg_edge[order]
        # boundaries per (core, win, grp)
        key = (co * nwin + wo) * ngrp + go
        cnt = np.bincount(key, minlength=NCORES * nwin * ngrp
                          ).reshape(NCORES, nwin, ngrp)
        starts = np.zeros_like(cnt)
        pos = 0
        # column offsets of each (win, grp) in the packed arrays (shared)
        colT = np.concatenate([[0], np.cumsum(T.ravel())])[:-1].reshape(T.shape)
        colS = np.concatenate([[0], np.cumsum(S.ravel())])[:-1].reshape(S.shape)
        e0 = 0
        for ci in range(NCORES):
            for w in range(nwin):
                for q in range(ngrp):
                    k = cnt[ci, w, q]
                    ids = lo_o[e0:e0 + k]
                    dls = dl_o[e0:e0 + k]
                    dgs = dg_o[e0:e0 + k]
                    e0 += k
                    t = int(T[w, q])
                    if t == 0:
                        continue
                    buf = np.full(t, zero_rows[q], np.int32)
                    buf[:k] = ids
                    base = int(colT[w, q]) // 16
                    idx_all[ci, :, base:base + t // 16] = (
                        buf.reshape(t // 16, 16).T)
                    db = np.full(((t + P - 1) // P) * P, 999.0, np.float32)
                    db[:k] = dls
                    gb = np.zeros(((t + P - 1) // P) * P, np.float32)
                    gb[:k] = dgs
                    sbase = int(colS[w, q])
                    ns = (t + P - 1) // P
                    dst_all[ci, :, sbase:sbase + ns] = (
                        db.reshape(ns, P).T)
                    dg_all[ci, :, sbase:sbase + ns] = (
                        gb.reshape(ns, P).T)
        assert e0 == len(order)
        # replicate idx 16-partition pattern to 128 partitions
        idx_rep = np.tile(idx_all, (1, 8, 1))
        return idx_rep, dst_all, dg_all, colT, colS, sumT, sumS

    zr1 = l1_blk_rows                          # zero row index per L1 block
    zr2 = l2_blk_rows                          # zero row index per L2 block
    idx1, dst1, dg1, colT1, colS1, sumT1, sumS1 = pack(g1, ngrp1, T1, S1, l1loc, zr1)
    idx2, dst2, dg2, colT2, colS2, sumT2, sumS2 = pack(c2, nchunk, T2, S2, l2loc, zr2)

    # x table with per-block zero row: block q rows [q*(rows+1) ... ]
    xblocks = []
    for q in range(ngrp1):
        xb = x[q * L1_RANGE: q * L1_RANGE + l1_blk_rows[q]]
        xblocks.append(np.concatenate([xb, np.zeros((1, x.shape[1]), np.float32)]))
    xdev = np.concatenate(xblocks, axis=0)
    l1_base = np.concatenate([[0], np.cumsum([b.shape[0] for b in xblocks])])[:-1]

    # per-core transposed shard + deg_inv layout
    xts, dinvs = [], []
    for ci in range(NCORES):
        xs = x[ci * shard:(ci + 1) * shard]
        pad = nwin * P - shard
        xts.append(np.concatenate(
            [xs, np.zeros((pad, x.shape[1]), np.float32)]).T.copy())
        dv = np.concatenate([deg_inv[ci * shard:(ci + 1) * shard],
                             np.zeros(pad, np.float32)])
        dinvs.append(dv.reshape(nwin, P).T.copy())

    meta = dict(T1=T1, T2=T2, S1=S1, S2=S2, colT1=colT1, colS1=colS1,
                colT2=colT2, colS2=colS2, sumT1=sumT1, sumS1=sumS1,
                sumT2=sumT2, sumS2=sumS2, l1_base=l1_base,
                l1_blk_rows=l1_blk_rows, l2_blk_rows=l2_blk_rows)
    data = dict(xdev=xdev, idx1=idx1, dst1=dst1, dg1=dg1, idx2=idx2,
                dst2=dst2, dg2=dg2, xts=xts, dinvs=dinvs)
    return meta, data


# ---------------------------------------------------------------- builder

def _build(cfg, meta, ablate=()):
    import concourse.bacc as bacc
    import concourse.mybir as mybir
    import concourse.tile as tile

    f32 = mybir.dt.float32
    shard, nwin, nchunk = cfg["shard"], cfg["nwin"], cfg["nchunk"]
    chunk_wins, chunk_rows, ngrp1 = cfg["chunk_wins"], cfg["chunk_rows"], cfg["ngrp1"]
    T1, T2, S1, S2 = meta["T1"], meta["T2"], meta["S1"], meta["S2"]
    colT1, colS1 = meta["colT1"], meta["colS1"]
    colT2, colS2 = meta["colT2"], meta["colS2"]
    l1_base = meta["l1_base"]
    l1_blk_rows, l2_blk_rows = meta["l1_blk_rows"], meta["l2_blk_rows"]
    S1w = S1.sum(axis=1)          # slices per window, layer 1
    S2w = S2.sum(axis=1)
    S1max, S2max = int(S1w.max()), int(S2w.max())
    xdev_rows = int(l1_base[-1] + l1_blk_rows[-1] + 1)

    # P2_full block offsets (each block followed by one zero row)
    p2_off = np.concatenate([[0], np.cumsum([r + 1 for r in l2_blk_rows])])
    p2_rows = int(p2_off[-1])

    nc = bacc.Bacc()
    dp = nc.declare_dram_parameter
    xdev = dp("xdev", [xdev_rows, IN_CH], f32, isOutput=False)
    xt = dp("xt", [P, nwin * P], f32, isOutput=False)
    idx1 = dp("idx1", [P, meta["sumT1"] // 16], mybir.dt.int16, isOutput=False)
    dst1 = dp("dst1", [P, meta["sumS1"]], f32, isOutput=False)
    dg1 = dp("dg1", [P, meta["sumS1"]], f32, isOutput=False)
    idx2 = dp("idx2", [P, meta["sumT2"] // 16], mybir.dt.int16, isOutput=False)
    dst2 = dp("dst2", [P, meta["sumS2"]], f32, isOutput=False)
    dg2 = dp("dg2", [P, meta["sumS2"]], f32, isOutput=False)
    w1l = dp("w1l", [IN_CH, HIDDEN], f32, isOutput=False)
    w1r = dp("w1r", [IN_CH, HIDDEN], f32, isOutput=False)
    w2l = dp("w2l", [HIDDEN, OUT_CH], f32, isOutput=False)
    w2r = dp("w2r", [HIDDEN, OUT_CH], f32, isOutput=False)
    b1c = dp("b1c", [P, 1], f32, isOutput=False)
    b2b = dp("b2b", [P, 1], f32, isOutput=False)
    iota = dp("iota", [P, P], f32, isOutput=False)
    ident = dp("ident", [P, P], f32, isOutput=False)
    y = dp("y", [OUT_CH, nwin * P], f32, isOutput=True)

    p2_full = nc.dram_tensor("p2_full", [p2_rows, OUT_CH], f32,
                             addr_space="Shared")

    with tile.TileContext(nc) as tc:
        with (
            tc.tile_pool(name="const", bufs=1) as cb,
            tc.tile_pool(name="sb", bufs=3) as sb,
            tc.tile_pool(name="ps", bufs=2, space="PSUM") as ps,
            tc.tile_pool(name="psb", bufs=1, space="PSUM") as psb,
            tc.tile_pool(name="dram", bufs=1, space="DRAM") as dr,
        ):
            # ---- constants
            def cload(param, shape, tag):
                t = cb.tile(shape, f32, tag=tag)
                nc.sync.dma_start(out=t[:], in_=param[:])
                return t
            iota_t = cload(iota, [P, P], "c_iota")
            ident_t = cload(ident, [P, P], "c_ident")
            w1l_t = cload(w1l, [IN_CH, HIDDEN], "c_w1l")
            w1r_t = cload(w1r, [IN_CH, HIDDEN], "c_w1r")
            w2l_t = cload(w2l, [HIDDEN, OUT_CH], "c_w2l")
            w2r_t = cload(w2r, [HIDDEN, OUT_CH], "c_w2r")
            b1_t = cload(b1c, [P, 1], "c_b1")
            b2_t = cload(b2b, [P, 1], "c_b2")
            r2_t = cb.tile([OUT_CH, nwin * P], f32)     # persistent R2 (transposed)
            zrow_t = cb.tile([P, OUT_CH], f32)
            nc.vector.memset(zrow_t[:], 0.0)

            # P2 chunk DRAM tiles (collective inputs)
            p2c = []
            for c in range(nchunk):
                p2c_tile = dr.tile([max(chunk_rows[c], 1), OUT_CH], f32,
                                   tag=f"p2c{c}")
                p2c.append(p2c_tile)

            # zero rows of p2_full (written once, before collectives run)
            for c in range(nchunk):
                zr = int(p2_off[c] + l2_blk_rows[c])
                nc.sync.dma_start(out=p2_full[zr:zr + 1, :], in_=zrow_t[:1, :])

            relu = mybir.ActivationFunctionType.Relu
            copyf = mybir.ActivationFunctionType.Copy

            # ---------------- phase 1 ----------------
            for w in range(nwin):
                n_w = min(shard - w * P, P)
                s1w = int(S1w[w])
                if s1w == 0:
                    continue
                # load idx/dstl/xt slices for this window
                it = sb.tile([P, int(T1[w].sum()) // 16], mybir.dt.int16, tag="it1")
                nc.sync.dma_start(
                    out=it[:], in_=idx1[:, int(colT1[w, 0]) // 16:
                                        (int(colT1[w, 0]) + int(T1[w].sum())) // 16])
                dt_ = sb.tile([P, s1w], f32, tag="dt1")
                nc.sync.dma_start(
                    out=dt_[:], in_=dst1[:, int(colS1[w, 0]):int(colS1[w, 0]) + s1w])
                dg_ = sb.tile([P, s1w], f32, tag="dg1")
                nc.sync.dma_start(
                    out=dg_[:], in_=dg1[:, int(colS1[w, 0]):int(colS1[w, 0]) + s1w])
                xtw = sb.tile([P, P], f32, tag="xtw")
                nc.sync.dma_start(out=xtw[:], in_=xt[:, w * P:(w + 1) * P])

                # gather slab
                gat = sb.tile([P, S1max * IN_CH], f32, tag="g1")
                nc.vector.memset(gat[:, :s1w * IN_CH], 0.0)
                for q in range(ngrp1):
                    t_q = int(T1[w, q])
                    if t_q == 0:
                        continue
                    cq = (t_q + P - 1) // P
                    sbase = int((S1[w, :q]).sum())
                    ibase = int(colT1[w, q] - colT1[w, 0]) // 16
                    blo = int(l1_base[q])
                    nrows = l1_blk_rows[q] + 1
                    if "nogather" in ablate:
                        continue
                    nc.gpsimd.dma_gather(
                        out_ap=gat[:, sbase * IN_CH:(sbase + cq) * IN_CH]
                        .rearrange("p (c e) -> p c e", e=IN_CH),
                        in_ap=xdev[blo:blo + nrows, :],
                        idxs_ap=it[:, ibase:ibase + t_q // 16],
                        num_idxs=t_q,
                        num_idxs_reg=t_q,
                        elem_size=IN_CH,
                        single_packet=False,
                    )

                # aggregation matmuls: psum1[f,n] += G_g^T @ M_g
                # (one-hot as moving operand; deginv folded into M)
                psum1 = ps.tile([P, IN_CH], f32, tag="ps1", space="PSUM")
                for g in range(s1w):
                    m = sb.tile([P, P], f32, tag="m1")
                    nc.vector.tensor_scalar(
                        out=m[:], in0=iota_t[:], scalar1=dt_[:, g:g + 1],
                        scalar2=dg_[:, g:g + 1],
                        op0=mybir.AluOpType.is_equal,
                        op1=mybir.AluOpType.mult)
                    nc.tensor.matmul(
                        out=psum1[:], lhsT=gat[:, g * IN_CH:(g + 1) * IN_CH],
                        rhs=m[:],
                        start=(g == 0), stop=(g == s1w - 1))

                # T1T = (D S1)^T  [f,n]
                t1t = sb.tile([P, P], f32, tag="t1t")
                nc.vector.tensor_copy(out=t1t[:], in_=psum1[:])
                # hT = relu(W1l^T T1T + W1r^T XTw + b1)  [h,n]
                psum2 = psb.tile([P, P], f32, tag="ps2", space="PSUM")
                nc.tensor.matmul(out=psum2[:], lhsT=w1l_t[:], rhs=t1t[:],
                                 start=True, stop=False)
                nc.tensor.matmul(out=psum2[:], lhsT=w1r_t[:], rhs=xtw[:],
                                 start=False, stop=True)
                ht = sb.tile([P, P], f32, tag="ht")
                nc.vector.tensor_scalar(
                    out=ht[:], in0=psum2[:], scalar1=b1_t[:, :1], scalar2=0.0,
                    op0=mybir.AluOpType.add, op1=mybir.AluOpType.max)
                # DMA-copy hT so it can be a stationary operand (lhsT)
                ht2 = sb.tile([P, P], f32, tag="ht2")
                nc.sync.dma_start(out=ht2[:], in_=ht[:])
                # P2 rows = h @ W2_l  [n,64]
                psum3 = psb.tile([P, OUT_CH], f32, tag="ps3", space="PSUM")
                nc.tensor.matmul(out=psum3[:], lhsT=ht2[:], rhs=w2l_t[:],
                                 start=True, stop=True)
                p2sb = sb.tile([P, OUT_CH], f32, tag="p2sb")
                nc.scalar.activation(out=p2sb[:], in_=psum3[:], func=copyf)
                c = min(w // chunk_wins, nchunk - 1)
                r0 = w * P - c * chunk_wins * P
                nc.sync.dma_start(out=p2c[c][r0:r0 + n_w, :], in_=p2sb[:n_w, :])
                # R2T = (h @ W2_r)^T + b2  [64,n] persistent
                psum4 = psb.tile([OUT_CH, P], f32, tag="ps4", space="PSUM")
                nc.tensor.matmul(out=psum4[:], lhsT=w2r_t[:], rhs=ht[:],
                                 start=True, stop=True)
                nc.vector.tensor_scalar(
                    out=r2_t[:, w * P:(w + 1) * P], in0=psum4[:],
                    scalar1=b2_t[:OUT_CH, :1], scalar2=None,
                    op0=mybir.AluOpType.add)

                # chunk AllGather once its windows are done
                if (w + 1) % chunk_wins == 0 or w == nwin - 1:
                    if (w + 1) % chunk_wins == 0:
                        c_done = (w + 1) // chunk_wins - 1
                    else:
                        c_done = nchunk - 1
                    off = int(p2_off[c_done])
                    rows = l2_blk_rows[c_done]
                    if "noag" in ablate:
                        continue
                    nc.gpsimd.collective_compute(
                        "AllGather",
                        mybir.AluOpType.bypass,
                        replica_groups=[list(range(NCORES))],
                        ins=[p2c[c_done].opt()],
                        outs=[p2_full[off:off + rows, :]],
                    )

            # ---------------- phase 2 ----------------
            for w in range(nwin):
                n_w = min(shard - w * P, P)
                s2w = int(S2w[w])
                if "nophase2" in ablate:
                    s2w = 0
                if s2w == 0:
                    # no edges into this window anywhere: y = R2
                    ysb = sb.tile([OUT_CH, P], f32, tag="ysb")
                    nc.vector.tensor_copy(
                        out=ysb[:], in_=r2_t[:, w * P:(w + 1) * P])
                    nc.sync.dma_start(out=y[:, w * P:(w + 1) * P], in_=ysb[:, :])
                    continue
                it = sb.tile([P, int(T2[w].sum()) // 16], mybir.dt.int16, tag="it2")
                nc.sync.dma_start(
                    out=it[:], in_=idx2[:, int(colT2[w, 0]) // 16:
                                        (int(colT2[w, 0]) + int(T2[w].sum())) // 16])
                dt_ = sb.tile([P, s2w], f32, tag="dt2")
                nc.sync.dma_start(
                    out=dt_[:], in_=dst2[:, int(colS2[w, 0]):int(colS2[w, 0]) + s2w])
                dg_ = sb.tile([P, s2w], f32, tag="dg2")
                nc.sync.dma_start(
                    out=dg_[:], in_=dg2[:, int(colS2[w, 0]):int(colS2[w, 0]) + s2w])
                gat = sb.tile([P, S2max * OUT_CH], f32, tag="g2")
                nc.vector.memset(gat[:, :s2w * OUT_CH], 0.0)
                for q in range(nchunk):
                    t_q = int(T2[w, q])
                    if t_q == 0:
                        continue
                    cq = (t_q + P - 1) // P
                    sbase = int((S2[w, :q]).sum())
                    ibase = int(colT2[w, q] - colT2[w, 0]) // 16
                    off = int(p2_off[q])
                    nrows = l2_blk_rows[q] + 1
                    if "nogather" in ablate:
                        continue
                    nc.gpsimd.dma_gather(
                        out_ap=gat[:, sbase * OUT_CH:(sbase + cq) * OUT_CH]
                        .rearrange("p (c e) -> p c e", e=OUT_CH),
                        in_ap=p2_full[off:off + nrows, :],
                        idxs_ap=it[:, ibase:ibase + t_q // 16],
                        num_idxs=t_q,
                        num_idxs_reg=t_q,
                        elem_size=OUT_CH,
                        single_packet=False,
                    )
                psum5 = ps.tile([OUT_CH, P], f32, tag="ps5", space="PSUM")
                for g in range(s2w):
                    m = sb.tile([P, P], f32, tag="m2")
                    nc.vector.tensor_scalar(
                        out=m[:], in0=iota_t[:], scalar1=dt_[:, g:g + 1],
                        scalar2=dg_[:, g:g + 1],
                        op0=mybir.AluOpType.is_equal,
                        op1=mybir.AluOpType.mult)
                    nc.tensor.matmul(
                        out=psum5[:], lhsT=gat[:, g * OUT_CH:(g + 1) * OUT_CH],
                        rhs=m[:],
                        start=(g == 0), stop=(g == s2w - 1))
                ysb = sb.tile([OUT_CH, P], f32, tag="ysb")
                nc.vector.tensor_add(out=ysb[:], in0=psum5[:],
                                     in1=r2_t[:, w * P:(w + 1) * P])
                nc.sync.dma_start(out=y[:, w * P:(w + 1) * P], in_=ysb[:, :])

    nc.compile()
    return nc


# ---------------------------------------------------------------- entry

_CACHE = {}


def kernel(x, edge_index, W1_l, W1_r, b1, W2_l, W2_r, b2):
    x = np.asarray(x, dtype=np.float32)
    edge_index = np.asarray(edge_index)
    cfg = _derive_cfg(x.shape[0])
    meta, data = _preprocess(x, edge_index, cfg)

    key = (x.shape, edge_index.shape)
    if key in _CACHE and _CACHE[key][1] == _meta_sig(meta):
        nc = _CACHE[key][0]
    else:
        nc = _build(cfg, meta)
        _CACHE[key] = (nc, _meta_sig(meta))

    in_maps = _make_inmaps(
        dict(W1_l=W1_l, W1_r=W1_r, b1=b1, W2_l=W2_l, W2_r=W2_r, b2=b2),
        meta, data)

    from concourse.bass_utils import run_bass_kernel_spmd
    r = run_bass_kernel_spmd(nc, in_maps, core_ids=list(range(NCORES)))
    shard = cfg["shard"]
    out = np.concatenate(
        [r.results[c]["y"].T[:shard] for c in range(NCORES)], axis=0)
    return np.ascontiguousarray(out, dtype=np.float32)


def _meta_sig(meta):
    return (int(meta["sumT1"]), int(meta["sumS1"]),
            int(meta["sumT2"]), int(meta["sumS2"]))


def _make_inmaps(inputs, meta, data):
    iota_v = np.tile(np.arange(P, dtype=np.float32), (P, 1))
    ident_v = np.eye(P, dtype=np.float32)
    common = dict(
        xdev=data["xdev"],
        w1l=np.asarray(inputs["W1_l"], np.float32),
        w1r=np.asarray(inputs["W1_r"], np.float32),
        w2l=np.asarray(inputs["W2_l"], np.float32),
        w2r=np.asarray(inputs["W2_r"], np.float32),
        b1c=np.asarray(inputs["b1"], np.float32).reshape(P, 1),
        b2b=np.concatenate([np.asarray(inputs["b2"], np.float32),
                            np.zeros(P - OUT_CH, np.float32)]).reshape(P, 1),
        iota=iota_v, ident=ident_v,
    )
    in_maps = []
    for ci in range(NCORES):
        m = dict(common)
        m["xt"] = data["xts"][ci]
        m["idx1"] = data["idx1"][ci]
        m["dst1"] = data["dst1"][ci]
        m["dg1"] = data["dg1"][ci]
        m["idx2"] = data["idx2"][ci]
        m["dst2"] = data["dst2"][ci]
        m["dg2"] = data["dg2"][ci]
        in_maps.append(m)
    return in_maps
